# revision 9
# baseline (speedup 1.0000x reference)
"""Trainium2 Bass kernel for a 2-layer LSTM decoder (B=512, T=128, H=1024).

Strategy (v2 — fp8 DoubleRow, 4 active cores x 128 batch rows):
  - The PE matmul cost is (output rows streamed) x cycles/row, independent of
    the stationary width M, so M must be 128 to use the full array. With
    batch-parallel sharding that requires 128 batch rows per core -> 4 cores
    carry the 512-row batch (cores 4-7 duplicate core c-4; their output is
    ignored).
  - All gate matmuls run as fp8e4m3 DoubleRow instructions: one instruction
    contracts K=256 (two 128-row k-tiles) at ~0.5 cycles per output row
    (measured 194 ns for [K=256]x[M=128]x[N=512] vs 216 ns for the bf16
    [K=128]x[M=64]x[N=512] unit -> ~4.4x per MAC).
  - Weights are pre-scaled by 16 and quantized to fp8 on the host; h is
    transposed per step (DMA transpose in bf16) then cast+scaled to fp8 on
    the scalar engine. PSUM holds gates x 256; activations descale by 1/256.
  - The x (scalar input) and bias contributions ride as one bf16 K=2 matmul
    per gate tile, with weights/biases pre-scaled by 256 on the host.
  - Output projection: fused DVE tensor_tensor_reduce (h1 * W_out, sum) with
    b_out as the reduction seed -> part [128,1]; a tiny PE transpose turns it
    into the next step's x row. Output rows accumulate in SBUF; one DMA at
    the end.
"""

import os

import numpy as np
import ml_dtypes

import concourse.bass as bass
import concourse.tile as tile
import concourse.mybir as mybir

BF16 = ml_dtypes.bfloat16
FP8 = ml_dtypes.float8_e4m3
N_CORES = 8
ACTIVE = 4
B, T_FULL, H = 512, 128, 1024
BL = B // ACTIVE  # 128 local batch rows
AF = mybir.ActivationFunctionType
DT = mybir.dt
DR = mybir.MatmulPerfMode.DoubleRow

SW = 16.0  # weight scale into fp8
SH = 16.0  # hidden-state scale into fp8
S2 = SW * SH

_T = int(os.environ.get("LSTM_KERNEL_T", str(T_FULL)))

# gate tile order (gate, half): i and g~ first so the c-chain starts early
TILE_ORDER = [(0, 0), (0, 1), (2, 0), (2, 1), (1, 0), (1, 1), (3, 0), (3, 1)]
GFUNC = {0: AF.Sigmoid, 1: AF.Sigmoid, 2: AF.Tanh, 3: AF.Sigmoid}


def _split_multi_waits(nc):
    """walrus in this container supports only ONE sync wait per instruction.
    Move extra waits onto preceding same-engine NOPs (engine FIFO makes this
    semantically identical)."""
    for f in nc.m.functions:
        for bb in f.blocks:
            new = []
            for ins in bb.instructions:
                si = ins.sync_info
                if si is not None and si.on_wait and len(si.on_wait) > 1:
                    waits = list(si.on_wait)
                    for w in waits[:-1]:
                        nop = mybir.InstNoOp(
                            name=nc.get_next_instruction_name(), ins=[], outs=[]
                        )
                        nop.engine = ins.engine
                        nop.sync_info = mybir.SyncInfo(on_wait=[w], on_update=[])
                        nc.register_instruction(nop)
                        new.append(nop)
                    si.on_wait = [waits[-1]]
                new.append(ins)
            bb.instructions = new


def _build_program(t_steps):
    nc = bass.Bass(dynamic_dma_scratch_size=512)

    w0_d = nc.dram_tensor("W0", [128, 4, 2, 4096], DT.float8e4, kind="ExternalInput")
    w1_d = nc.dram_tensor("W1", [128, 8, 2, 4096], DT.float8e4, kind="ExternalInput")
    wxb0_d = nc.dram_tensor("WXB0", [2, 4096], DT.bfloat16, kind="ExternalInput")
    wb1_d = nc.dram_tensor("WB1", [1, 4096], DT.bfloat16, kind="ExternalInput")
    woutb_d = nc.dram_tensor("WOUTB", [128, 1024], DT.bfloat16, kind="ExternalInput")
    boutb_d = nc.dram_tensor("BOUTB", [128, 1], DT.float32, kind="ExternalInput")
    idt_d = nc.dram_tensor("IDT", [128, 128], DT.bfloat16, kind="ExternalInput")
    ht8_d = nc.dram_tensor("HT8", [128, 2, 8, 128], DT.float8e4, kind="ExternalInput")
    cs_d = nc.dram_tensor("CS", [128, 2, 1024], DT.float32, kind="ExternalInput")
    xo0_d = nc.dram_tensor("XO0", [2, 128], DT.bfloat16, kind="ExternalInput")
    outb_d = nc.dram_tensor("OUTB", [128, T_FULL], DT.float32, kind="ExternalOutput")

    with tile.TileContext(nc) as tc:
        with (
            tc.tile_pool(name="const", bufs=1) as const,
            tc.tile_pool(name="psum", bufs=8, space="PSUM") as psum,
        ):
            w0 = const.tile([128, 4, 2, 4096], DT.float8e4)
            w1 = const.tile([128, 8, 2, 4096], DT.float8e4)
            wxb0 = const.tile([2, 4096], DT.bfloat16)
            wb1 = const.tile([1, 4096], DT.bfloat16)
            woutb = const.tile([128, 1024], DT.bfloat16)
            boutb = const.tile([128, 1], DT.float32)
            idt = const.tile([128, 128], DT.bfloat16)
            ht8 = const.tile([128, 2, 8, 128], DT.float8e4)
            cs = const.tile([128, 2, 1024], DT.float32)
            xo = const.tile([2, 128], DT.bfloat16)
            ones = const.tile([1, 128], DT.bfloat16)
            ga0 = const.tile([128, 4, 1024], DT.float32)
            ga1 = const.tile([128, 4, 1024], DT.float32)
            t10 = const.tile([128, 1024], DT.float32)
            t11 = const.tile([128, 1024], DT.float32)
            thc0 = const.tile([128, 1024], DT.float32)
            thc1 = const.tile([128, 1024], DT.float32)
            hs0 = const.tile([128, 1024], DT.bfloat16)
            hs1 = const.tile([128, 1024], DT.bfloat16)
            htb0 = const.tile([128, 8, 128], DT.bfloat16)
            htb1 = const.tile([128, 8, 128], DT.bfloat16)
            scr = const.tile([128, 1024], DT.float32)
            part = const.tile([128, 1], DT.float32)
            partb = const.tile([128, 1], DT.bfloat16)
            outbuf = const.tile([128, T_FULL], DT.float32)

            nc.sync.dma_start(w0[:], w0_d[:])
            nc.sync.dma_start(w1[:], w1_d[:])
            nc.sync.dma_start(wxb0[:], wxb0_d[:])
            nc.sync.dma_start(wb1[:], wb1_d[:])
            nc.sync.dma_start(woutb[:], woutb_d[:])
            nc.sync.dma_start(boutb[:], boutb_d[:])
            nc.sync.dma_start(idt[:], idt_d[:])
            nc.sync.dma_start(ht8[:], ht8_d[:])
            nc.sync.dma_start(cs[:], cs_d[:])
            nc.sync.dma_start(xo[:], xo0_d[:])
            nc.vector.memset(ones[:], 1.0)
            nc.vector.memset(outbuf[:], 0.0)

            for t in range(t_steps):
                # ---- L0 gate DR matmuls (tiles 0-3), using h0(t-1), then the
                # x-row transpose (x(t) = out(t-1)), then tiles 4-7.
                p0 = {}
                half = TILE_ORDER[:4]
                rest = TILE_ORDER[4:]
                for g, hf in half:
                    ptile = psum.tile([128, 512], DT.float32, tag="bank",
                                      name=f"p0_{t}_{g}{hf}")
                    p0[(g, hf)] = ptile
                    s = 1024 * g + 512 * hf
                    for p in range(4):
                        nc.tensor.matmul(
                            ptile[:, :],
                            ht8[:, 0, 2 * p : 2 * p + 2, :],
                            w0[:, p, :, s : s + 512],
                            start=(p == 0), stop=False, perf_mode=DR,
                        )
                if t > 0:
                    xps = psum.tile([1, 128], DT.bfloat16, tag="bank",
                                    name=f"xps_{t}")
                    nc.tensor.transpose(xps[0:1, :], partb[:, 0:1], idt[:])
                    nc.scalar.activation(xo[0:1, :], xps[0:1, :], AF.Copy)
                for g, hf in rest:
                    ptile = psum.tile([128, 512], DT.float32, tag="bank",
                                      name=f"p0_{t}_{g}{hf}")
                    p0[(g, hf)] = ptile
                    s = 1024 * g + 512 * hf
                    for p in range(4):
                        nc.tensor.matmul(
                            ptile[:, :],
                            ht8[:, 0, 2 * p : 2 * p + 2, :],
                            w0[:, p, :, s : s + 512],
                            start=(p == 0), stop=False, perf_mode=DR,
                        )
                # x + bias rider (bf16, K=2), closes each group
                for g, hf in TILE_ORDER:
                    s = 1024 * g + 512 * hf
                    nc.tensor.matmul(
                        p0[(g, hf)][:, :], xo[0:2, :], wxb0[0:2, s : s + 512],
                        start=False, stop=True,
                    )

                # ---- L0 activations (descale by 1/S2) + c-chain
                for g, hf in TILE_ORDER:
                    nc.scalar.activation(
                        ga0[:, g, 512 * hf : 512 * hf + 512],
                        p0[(g, hf)][:, :], GFUNC[g], scale=1.0 / S2,
                    )
                cs0 = cs[:, 0, :]
                nc.vector.tensor_mul(t10[:], ga0[:, 0, :], ga0[:, 2, :])
                nc.vector.tensor_mul(cs0, ga0[:, 1, :], cs0)
                nc.vector.tensor_add(cs0, cs0, t10[:])
                nc.scalar.activation(thc0[:], cs0, AF.Tanh)
                nc.vector.tensor_mul(hs0[:], ga0[:, 3, :], thc0[:])

                # ---- transpose h0 -> htb0 (DMA xbar), cast+scale to fp8
                for k in range(8):
                    eng = nc.scalar if k % 2 == 0 else nc.sync
                    eng.dma_start_transpose(
                        htb0[:, k, :], hs0[:, 128 * k : 128 * (k + 1)]
                    )
                nc.scalar.activation(ht8[:, 0, :, :], htb0[:, :, :], AF.Copy,
                                     scale=SH)

                # ---- L1: Whh1 (h1(t-1)) then Wih1 (h0(t)) then bias
                p1 = {}
                for g, hf in TILE_ORDER:
                    ptile = psum.tile([128, 512], DT.float32, tag="bank",
                                      name=f"p1_{t}_{g}{hf}")
                    p1[(g, hf)] = ptile
                    s = 1024 * g + 512 * hf
                    for p in range(4):
                        nc.tensor.matmul(
                            ptile[:, :],
                            ht8[:, 1, 2 * p : 2 * p + 2, :],
                            w1[:, p, :, s : s + 512],
                            start=(p == 0), stop=False, perf_mode=DR,
                        )
                for g, hf in TILE_ORDER:
                    s = 1024 * g + 512 * hf
                    for p in range(4):
                        nc.tensor.matmul(
                            p1[(g, hf)][:, :],
                            ht8[:, 0, 2 * p : 2 * p + 2, :],
                            w1[:, 4 + p, :, s : s + 512],
                            start=False, stop=False, perf_mode=DR,
                        )
                for g, hf in TILE_ORDER:
                    s = 1024 * g + 512 * hf
                    nc.tensor.matmul(
                        p1[(g, hf)][:, :], ones[0:1, :], wb1[0:1, s : s + 512],
                        start=False, stop=True,
                    )

                # ---- L1 activations + c-chain
                for g, hf in TILE_ORDER:
                    nc.scalar.activation(
                        ga1[:, g, 512 * hf : 512 * hf + 512],
                        p1[(g, hf)][:, :], GFUNC[g], scale=1.0 / S2,
                    )
                cs1 = cs[:, 1, :]
                nc.vector.tensor_mul(t11[:], ga1[:, 0, :], ga1[:, 2, :])
                nc.vector.tensor_mul(cs1, ga1[:, 1, :], cs1)
                nc.vector.tensor_add(cs1, cs1, t11[:])
                nc.scalar.activation(thc1[:], cs1, AF.Tanh)
                nc.vector.tensor_mul(hs1[:], ga1[:, 3, :], thc1[:])

                # ---- transpose h1 -> htb1, cast to fp8
                for k in range(8):
                    eng = nc.scalar if k % 2 == 0 else nc.sync
                    eng.dma_start_transpose(
                        htb1[:, k, :], hs1[:, 128 * k : 128 * (k + 1)]
                    )
                nc.scalar.activation(ht8[:, 1, :, :], htb1[:, :, :], AF.Copy,
                                     scale=SH)

                # ---- out = W_out . h1 + b_out via fused mul-reduce
                nc.vector.tensor_mul(scr[:], hs1[:], woutb[:])
                nc.vector.tensor_reduce(
                    out=part[:, 0:1], in_=scr[:], op=mybir.AluOpType.add,
                    axis=mybir.AxisListType.X,
                )
                nc.scalar.activation(partb[:, 0:1], part[:, 0:1], AF.Identity,
                                     bias=boutb[:, 0:1])
                tw = t % T_FULL
                nc.scalar.activation(outbuf[:, tw : tw + 1], part[:, 0:1],
                                     AF.Identity, bias=boutb[:, 0:1])

            nc.sync.dma_start(outb_d[:], outbuf[:])

    _split_multi_waits(nc)
    return nc


# ---------------------------------------------------------------------------
# host side


def _prep_shared(inp):
    def pack_pairs(wT, npairs):
        # wT: [1024, 4096] (k, gate-dim) -> [128, npairs, 2, 4096]
        return (
            wT.reshape(npairs, 2, 128, 4096).transpose(2, 0, 1, 3)
        )

    whh0T = np.array(inp["W_hh0"], np.float32).T  # [1024, 4096]
    whh1T = np.array(inp["W_hh1"], np.float32).T
    wih1T = np.array(inp["W_ih1"], np.float32).T

    w0 = pack_pairs(whh0T * SW, 4)  # [128, 2, 4, 4096]
    w1 = np.concatenate(
        [pack_pairs(whh1T * SW, 4), pack_pairs(wih1T * SW, 4)], axis=1
    )  # [128, 8, 2, 4096]

    wxb0 = np.stack(
        [
            np.array(inp["W_ih0"], np.float32)[:, 0] * S2,
            (np.array(inp["b_ih0"], np.float32)
             + np.array(inp["b_hh0"], np.float32)) * S2,
        ]
    )  # [2, 4096]
    wb1 = ((np.array(inp["b_ih1"], np.float32)
            + np.array(inp["b_hh1"], np.float32)) * S2)[None, :]

    woutb = np.repeat(np.array(inp["W_out"], np.float32), 128, axis=0)
    boutb = np.full((128, 1), float(np.array(inp["b_out"])[0]), np.float32)

    xo0 = np.zeros((2, 128), np.float32)
    xo0[1] = 1.0

    return {
        "W0": w0.astype(FP8),
        "W1": w1.astype(FP8),
        "WXB0": wxb0.astype(BF16),
        "WB1": wb1.astype(BF16),
        "WOUTB": woutb.astype(BF16),
        "BOUTB": boutb,
        "IDT": np.eye(128, dtype=BF16),
        "XO0": xo0.astype(BF16),
    }


def _prep_core(inp, c):
    sl = slice(BL * (c % ACTIVE), BL * (c % ACTIVE + 1))
    ht8 = np.zeros((128, 2, 8, 128), np.float32)
    for l in range(2):
        hT = np.array(inp["h0"][l, sl], np.float32).T  # [1024, 128]
        ht8[:, l] = hT.reshape(8, 128, 128).transpose(1, 0, 2) * SH
    cs = np.stack(
        [np.array(inp["c0"][0, sl], np.float32),
         np.array(inp["c0"][1, sl], np.float32)], axis=1
    )  # [128, 2, 1024]
    return {"HT8": ht8.astype(FP8), "CS": np.ascontiguousarray(cs)}


_RUNNER = {}


def _get_runner(t_steps):
    """Build the bass program once per process and return a cached callable
    mapping per-core input dicts -> per-core OUTB arrays."""
    if t_steps in _RUNNER:
        return _RUNNER[t_steps]

    import jax
    from jax.sharding import Mesh, PartitionSpec
    from jax.experimental.shard_map import shard_map
    from concourse import bass2jax
    from concourse._compat import axon_active

    nc = _build_program(t_steps)

    if not axon_active():
        from concourse.bass_utils import run_bass_kernel_spmd

        def run_native(in_maps):
            res = run_bass_kernel_spmd(nc, in_maps, list(range(N_CORES)))
            return [r["OUTB"] for r in res.results]

        _RUNNER[t_steps] = run_native
        return run_native

    bass2jax.install_neuronx_cc_hook()

    partition_name = nc.partition_id_tensor.name if nc.partition_id_tensor else None
    in_names = []
    out_names = []
    out_avals = []
    zero_outs = []
    for alloc in nc.m.functions[0].allocations:
        if not isinstance(alloc, mybir.MemoryLocationSet):
            continue
        name = alloc.memorylocations[0].name
        if alloc.kind == "ExternalInput":
            if name != partition_name:
                in_names.append(name)
        elif alloc.kind == "ExternalOutput":
            out_names.append(name)
            shape = tuple(alloc.tensor_shape)
            dtype = mybir.dt.np(alloc.dtype)
            out_avals.append(jax.core.ShapedArray(shape, dtype))
            zero_outs.append(np.zeros(shape, dtype))
    n_params = len(in_names)
    n_outs = len(out_avals)
    all_names = in_names + out_names
    if partition_name is not None:
        all_names = all_names + [partition_name]
    donate = tuple(range(n_params, n_params + n_outs))

    def _body(*args):
        operands = list(args)
        if partition_name is not None:
            operands.append(bass2jax.partition_id_tensor())
        outs = bass2jax._bass_exec_p.bind(
            *operands,
            out_avals=tuple(out_avals),
            in_names=tuple(all_names),
            out_names=tuple(out_names),
            lowering_input_output_aliases=(),
            sim_require_finite=True,
            sim_require_nnan=True,
            nc=nc,
        )
        return tuple(outs)

    devices = jax.devices()[:N_CORES]
    mesh = Mesh(np.asarray(devices), ("core",))
    sharded = jax.jit(
        shard_map(
            _body,
            mesh=mesh,
            in_specs=(PartitionSpec("core"),) * (n_params + n_outs),
            out_specs=(PartitionSpec("core"),) * n_outs,
            check_rep=False,
        ),
        donate_argnums=donate,
        keep_unused=True,
    )

    def prep_args(in_maps):
        concat_in = [
            np.concatenate([np.asarray(in_maps[c][nm]) for c in range(N_CORES)], axis=0)
            for nm in in_names
        ]
        concat_zero = [np.concatenate([z] * N_CORES, axis=0) for z in zero_outs]
        return concat_in, concat_zero

    def run(in_maps):
        concat_in, concat_zero = prep_args(in_maps)
        out_arrs = sharded(*concat_in, *concat_zero)
        full = np.asarray(out_arrs[0])
        return np.split(full, N_CORES, axis=0)

    run.sharded = sharded
    run.prep_args = prep_args
    run.mesh = mesh
    _RUNNER[t_steps] = run
    return run


def kernel(**inputs):
    inp = {k: np.asarray(v) for k, v in inputs.items()}
    for k in ("W_ih0", "W_hh0", "b_ih0", "b_hh0", "W_ih1", "W_hh1", "b_ih1",
              "b_hh1", "W_out", "b_out", "h0", "c0", "outputs"):
        assert k in inp, f"missing input {k}"

    shared = _prep_shared(inp)
    in_maps = []
    for c in range(N_CORES):
        m = dict(shared)
        m.update(_prep_core(inp, c))
        in_maps.append(m)

    run = _get_runner(_T)
    outs = run(in_maps)  # list of [BL, T_FULL] fp32 per core

    out_all = np.concatenate(outs[:ACTIVE], axis=0)  # [B, T_FULL]
    targets = np.asarray(inp["outputs"]).astype(np.float32)  # [B, T]
    d = out_all[:, :_T].astype(np.float64) - targets[:, :_T].astype(np.float64)
    loss = np.sum(np.mean(d * d, axis=0))
    return np.float32(loss)


# revision 12
# speedup vs baseline: 1.4834x; 1.4834x over previous
"""Trainium2 Bass kernel for a 2-layer LSTM decoder (B=512, T=128, H=1024).

Strategy (v2 — fp8 DoubleRow, 4 active cores x 128 batch rows):
  - The PE matmul cost is (output rows streamed) x cycles/row, independent of
    the stationary width M, so M must be 128 to use the full array. With
    batch-parallel sharding that requires 128 batch rows per core -> 4 cores
    carry the 512-row batch (cores 4-7 duplicate core c-4; their output is
    ignored).
  - All gate matmuls run as fp8e4m3 DoubleRow instructions: one instruction
    contracts K=256 (two 128-row k-tiles) at ~0.5 cycles per output row
    (measured 194 ns for [K=256]x[M=128]x[N=512] vs 216 ns for the bf16
    [K=128]x[M=64]x[N=512] unit -> ~4.4x per MAC).
  - Weights are pre-scaled by 16 and quantized to fp8 on the host; h is
    transposed per step (DMA transpose in bf16) then cast+scaled to fp8 on
    the scalar engine. PSUM holds gates x 256; activations descale by 1/256.
  - The x (scalar input) and bias contributions ride as one bf16 K=2 matmul
    per gate tile, with weights/biases pre-scaled by 256 on the host.
  - Output projection: fused DVE tensor_tensor_reduce (h1 * W_out, sum) with
    b_out as the reduction seed -> part [128,1]; a tiny PE transpose turns it
    into the next step's x row. Output rows accumulate in SBUF; one DMA at
    the end.
"""

import os

import numpy as np
import ml_dtypes

import concourse.bass as bass
import concourse.tile as tile
import concourse.mybir as mybir

BF16 = ml_dtypes.bfloat16
FP8 = ml_dtypes.float8_e4m3
N_CORES = 8
ACTIVE = 4
B, T_FULL, H = 512, 128, 1024
BL = B // ACTIVE  # 128 local batch rows
AF = mybir.ActivationFunctionType
DT = mybir.dt
DR = mybir.MatmulPerfMode.DoubleRow

SW = 16.0  # weight scale into fp8
SH = 16.0  # hidden-state scale into fp8
S2 = SW * SH

_T = int(os.environ.get("LSTM_KERNEL_T", str(T_FULL)))

# gate tile order (gate, half): i and g~ first so the c-chain starts early
TILE_ORDER = [(0, 0), (0, 1), (2, 0), (2, 1), (1, 0), (1, 1), (3, 0), (3, 1)]
GFUNC = {0: AF.Sigmoid, 1: AF.Sigmoid, 2: AF.Tanh, 3: AF.Sigmoid}


def _split_multi_waits(nc):
    """walrus in this container supports only ONE sync wait per instruction.
    Move extra waits onto preceding same-engine NOPs (engine FIFO makes this
    semantically identical)."""
    for f in nc.m.functions:
        for bb in f.blocks:
            new = []
            for ins in bb.instructions:
                si = ins.sync_info
                if si is not None and si.on_wait and len(si.on_wait) > 1:
                    waits = list(si.on_wait)
                    for w in waits[:-1]:
                        nop = mybir.InstNoOp(
                            name=nc.get_next_instruction_name(), ins=[], outs=[]
                        )
                        nop.engine = ins.engine
                        nop.sync_info = mybir.SyncInfo(on_wait=[w], on_update=[])
                        nc.register_instruction(nop)
                        new.append(nop)
                    si.on_wait = [waits[-1]]
                new.append(ins)
            bb.instructions = new


def _build_program(t_steps):
    nc = bass.Bass(dynamic_dma_scratch_size=512)

    w0_d = nc.dram_tensor("W0", [128, 4, 2, 4096], DT.float8e4, kind="ExternalInput")
    w1_d = nc.dram_tensor("W1", [128, 8, 2, 4096], DT.float8e4, kind="ExternalInput")
    wxb0_d = nc.dram_tensor("WXB0", [2, 4096], DT.bfloat16, kind="ExternalInput")
    wb1_d = nc.dram_tensor("WB1", [1, 4096], DT.bfloat16, kind="ExternalInput")
    woutb_d = nc.dram_tensor("WOUTB", [128, 1024], DT.bfloat16, kind="ExternalInput")
    boutb_d = nc.dram_tensor("BOUTB", [128, 1], DT.float32, kind="ExternalInput")
    ht8_d = nc.dram_tensor("HT8", [128, 2, 8, 128], DT.float8e4, kind="ExternalInput")
    cs_d = nc.dram_tensor("CS", [128, 2, 1024], DT.float32, kind="ExternalInput")
    xo0_d = nc.dram_tensor("XO0", [128, 128], DT.bfloat16, kind="ExternalInput")
    outb_d = nc.dram_tensor("OUTB", [128, T_FULL], DT.float32, kind="ExternalOutput")

    with tile.TileContext(nc) as tc:
        with (
            tc.tile_pool(name="const", bufs=1) as const,
            tc.tile_pool(name="psum", bufs=8, space="PSUM") as psum,
        ):
            w0 = const.tile([128, 4, 2, 4096], DT.float8e4)
            w1 = const.tile([128, 8, 2, 4096], DT.float8e4)
            wxb0 = const.tile([2, 4096], DT.bfloat16)
            wb1 = const.tile([1, 4096], DT.bfloat16)
            woutb = const.tile([128, 1024], DT.bfloat16)
            boutb = const.tile([128, 1], DT.float32)
            ht8 = const.tile([128, 2, 8, 128], DT.float8e4)
            cs = const.tile([128, 2, 1024], DT.float32)
            xrow32 = const.tile([128, 128], DT.bfloat16)
            partb32 = const.tile([128, 128], DT.bfloat16)
            ones = const.tile([1, 128], DT.bfloat16)
            ga0 = const.tile([128, 4, 1024], DT.float32)
            ga1 = const.tile([128, 4, 1024], DT.float32)
            t10 = const.tile([128, 1024], DT.float32)
            t11 = const.tile([128, 1024], DT.float32)
            thc0 = const.tile([128, 1024], DT.float32)
            thc1 = const.tile([128, 1024], DT.float32)
            hs0 = const.tile([128, 1024], DT.bfloat16)
            hs1 = const.tile([128, 1024], DT.bfloat16)
            htb0 = const.tile([128, 8, 128], DT.bfloat16)
            htb1 = const.tile([128, 8, 128], DT.bfloat16)
            scr = const.tile([128, 1024], DT.float32)
            part = const.tile([128, 1], DT.float32)
            outbuf = const.tile([128, T_FULL], DT.float32)

            nc.sync.dma_start(w0[:], w0_d[:])
            nc.sync.dma_start(w1[:], w1_d[:])
            nc.sync.dma_start(wxb0[:], wxb0_d[:])
            nc.sync.dma_start(wb1[:], wb1_d[:])
            nc.sync.dma_start(woutb[:], woutb_d[:])
            nc.sync.dma_start(boutb[:], boutb_d[:])
            nc.sync.dma_start(ht8[:], ht8_d[:])
            nc.sync.dma_start(cs[:], cs_d[:])
            nc.sync.dma_start(xrow32[:], xo0_d[:])
            nc.vector.memset(partb32[:], 1.0)
            nc.vector.memset(ones[:], 1.0)
            nc.vector.memset(outbuf[:], 0.0)

            for t in range(t_steps):
                # ---- L0 gate DR matmuls, k-pair outer / tile inner so
                # consecutive instructions hit different PSUM banks and the
                # stationary h^T pair is reused across 8 instructions.
                p0 = {}
                for g, hf in TILE_ORDER:
                    p0[(g, hf)] = psum.tile([128, 512], DT.float32, tag="bank",
                                            name=f"p0_{t}_{g}{hf}")
                for p in range(4):
                    for g, hf in TILE_ORDER:
                        s = 1024 * g + 512 * hf
                        nc.tensor.matmul(
                            p0[(g, hf)][:, :],
                            ht8[:, 0, 2 * p : 2 * p + 2, :],
                            w0[:, p, :, s : s + 512],
                            start=(p == 0), stop=False, perf_mode=DR,
                        )
                # x + bias rider (bf16, K=2: [x; ones] rows of xrow32)
                for g, hf in TILE_ORDER:
                    s = 1024 * g + 512 * hf
                    nc.tensor.matmul(
                        p0[(g, hf)][:, :], xrow32[0:2, :],
                        wxb0[0:2, s : s + 512],
                        start=False, stop=True,
                    )

                # ---- L0 activations (descale by 1/S2) + c-chain
                for g, hf in TILE_ORDER:
                    nc.scalar.activation(
                        ga0[:, g, 512 * hf : 512 * hf + 512],
                        p0[(g, hf)][:, :], GFUNC[g], scale=1.0 / S2,
                    )
                cs0 = cs[:, 0, :]
                nc.vector.tensor_mul(t10[:], ga0[:, 0, :], ga0[:, 2, :])
                nc.vector.tensor_mul(cs0, ga0[:, 1, :], cs0)
                nc.vector.tensor_add(cs0, cs0, t10[:])
                nc.scalar.activation(thc0[:], cs0, AF.Tanh)
                nc.vector.tensor_mul(hs0[:], ga0[:, 3, :], thc0[:])

                # ---- L1e (Whh1 on h1(t-1)): only needs last step's ht8[:,1]
                p1 = {}
                for g, hf in TILE_ORDER:
                    p1[(g, hf)] = psum.tile([128, 512], DT.float32, tag="bank",
                                            name=f"p1_{t}_{g}{hf}")
                for p in range(4):
                    for g, hf in TILE_ORDER:
                        s = 1024 * g + 512 * hf
                        nc.tensor.matmul(
                            p1[(g, hf)][:, :],
                            ht8[:, 1, 2 * p : 2 * p + 2, :],
                            w1[:, p, :, s : s + 512],
                            start=(p == 0), stop=False, perf_mode=DR,
                        )

                # ---- h0 transpose -> cast -> L1l, pipelined per k-pair:
                # the pair-p matmuls only need h^T chunks 2p, 2p+1.
                for p in range(4):
                    nc.scalar.dma_start_transpose(
                        htb0[:, 2 * p, :],
                        hs0[:, 256 * p : 256 * p + 128],
                    )
                    nc.sync.dma_start_transpose(
                        htb0[:, 2 * p + 1, :],
                        hs0[:, 256 * p + 128 : 256 * p + 256],
                    )
                    nc.vector.tensor_scalar_mul(
                        ht8[:, 0, 2 * p : 2 * p + 2, :],
                        htb0[:, 2 * p : 2 * p + 2, :], SH,
                    )
                    for g, hf in TILE_ORDER:
                        s = 1024 * g + 512 * hf
                        nc.tensor.matmul(
                            p1[(g, hf)][:, :],
                            ht8[:, 0, 2 * p : 2 * p + 2, :],
                            w1[:, 4 + p, :, s : s + 512],
                            start=False, stop=False, perf_mode=DR,
                        )
                for g, hf in TILE_ORDER:
                    s = 1024 * g + 512 * hf
                    nc.tensor.matmul(
                        p1[(g, hf)][:, :], ones[0:1, :], wb1[0:1, s : s + 512],
                        start=False, stop=True,
                    )

                # ---- L1 activations + c-chain
                for g, hf in TILE_ORDER:
                    nc.scalar.activation(
                        ga1[:, g, 512 * hf : 512 * hf + 512],
                        p1[(g, hf)][:, :], GFUNC[g], scale=1.0 / S2,
                    )
                cs1 = cs[:, 1, :]
                nc.vector.tensor_mul(t11[:], ga1[:, 0, :], ga1[:, 2, :])
                nc.vector.tensor_mul(cs1, ga1[:, 1, :], cs1)
                nc.vector.tensor_add(cs1, cs1, t11[:])
                nc.scalar.activation(thc1[:], cs1, AF.Tanh)
                nc.vector.tensor_mul(hs1[:], ga1[:, 3, :], thc1[:])

                # ---- out = W_out . h1 + b_out; write next x into partb32
                # col 0 (cols 1-31 are static ones), DMA-transpose it to the
                # [x; ones] row pair the riders consume.
                nc.vector.tensor_mul(scr[:], hs1[:], woutb[:])
                nc.vector.tensor_reduce(
                    out=part[:, 0:1], in_=scr[:], op=mybir.AluOpType.add,
                    axis=mybir.AxisListType.X,
                )
                nc.scalar.activation(partb32[:, 0:1], part[:, 0:1],
                                     AF.Identity, bias=boutb[:, 0:1])
                tw = t % T_FULL
                nc.scalar.activation(outbuf[:, tw : tw + 1], part[:, 0:1],
                                     AF.Identity, bias=boutb[:, 0:1])
                if t + 1 < t_steps:
                    nc.sync.dma_start_transpose(xrow32[:, :], partb32[:, :])

                # ---- h1 transpose + cast (feeds next step's L1e; has slack)
                for p in range(4):
                    nc.scalar.dma_start_transpose(
                        htb1[:, 2 * p, :],
                        hs1[:, 256 * p : 256 * p + 128],
                    )
                    nc.sync.dma_start_transpose(
                        htb1[:, 2 * p + 1, :],
                        hs1[:, 256 * p + 128 : 256 * p + 256],
                    )
                    nc.vector.tensor_scalar_mul(
                        ht8[:, 1, 2 * p : 2 * p + 2, :],
                        htb1[:, 2 * p : 2 * p + 2, :], SH,
                    )

            nc.sync.dma_start(outb_d[:], outbuf[:])

    _split_multi_waits(nc)
    return nc


# ---------------------------------------------------------------------------
# host side


def _prep_shared(inp):
    def pack_pairs(wT, npairs):
        # wT: [1024, 4096] (k, gate-dim) -> [128, npairs, 2, 4096]
        return (
            wT.reshape(npairs, 2, 128, 4096).transpose(2, 0, 1, 3)
        )

    whh0T = np.array(inp["W_hh0"], np.float32).T  # [1024, 4096]
    whh1T = np.array(inp["W_hh1"], np.float32).T
    wih1T = np.array(inp["W_ih1"], np.float32).T

    w0 = pack_pairs(whh0T * SW, 4)  # [128, 2, 4, 4096]
    w1 = np.concatenate(
        [pack_pairs(whh1T * SW, 4), pack_pairs(wih1T * SW, 4)], axis=1
    )  # [128, 8, 2, 4096]

    wxb0 = np.stack(
        [
            np.array(inp["W_ih0"], np.float32)[:, 0] * S2,
            (np.array(inp["b_ih0"], np.float32)
             + np.array(inp["b_hh0"], np.float32)) * S2,
        ]
    )  # [2, 4096]
    wb1 = ((np.array(inp["b_ih1"], np.float32)
            + np.array(inp["b_hh1"], np.float32)) * S2)[None, :]

    woutb = np.repeat(np.array(inp["W_out"], np.float32), 128, axis=0)
    boutb = np.full((128, 1), float(np.array(inp["b_out"])[0]), np.float32)

    xo0 = np.ones((128, 128), np.float32)
    xo0[0] = 0.0

    return {
        "W0": w0.astype(FP8),
        "W1": w1.astype(FP8),
        "WXB0": wxb0.astype(BF16),
        "WB1": wb1.astype(BF16),
        "WOUTB": woutb.astype(BF16),
        "BOUTB": boutb,
        "XO0": xo0.astype(BF16),
    }


def _prep_core(inp, c):
    sl = slice(BL * (c % ACTIVE), BL * (c % ACTIVE + 1))
    ht8 = np.zeros((128, 2, 8, 128), np.float32)
    for l in range(2):
        hT = np.array(inp["h0"][l, sl], np.float32).T  # [1024, 128]
        ht8[:, l] = hT.reshape(8, 128, 128).transpose(1, 0, 2) * SH
    cs = np.stack(
        [np.array(inp["c0"][0, sl], np.float32),
         np.array(inp["c0"][1, sl], np.float32)], axis=1
    )  # [128, 2, 1024]
    return {"HT8": ht8.astype(FP8), "CS": np.ascontiguousarray(cs)}


_RUNNER = {}


def _get_runner(t_steps):
    """Build the bass program once per process and return a cached callable
    mapping per-core input dicts -> per-core OUTB arrays."""
    if t_steps in _RUNNER:
        return _RUNNER[t_steps]

    import jax
    from jax.sharding import Mesh, PartitionSpec
    from jax.experimental.shard_map import shard_map
    from concourse import bass2jax
    from concourse._compat import axon_active

    nc = _build_program(t_steps)

    if not axon_active():
        from concourse.bass_utils import run_bass_kernel_spmd

        def run_native(in_maps):
            res = run_bass_kernel_spmd(nc, in_maps, list(range(N_CORES)))
            return [r["OUTB"] for r in res.results]

        _RUNNER[t_steps] = run_native
        return run_native

    bass2jax.install_neuronx_cc_hook()

    partition_name = nc.partition_id_tensor.name if nc.partition_id_tensor else None
    in_names = []
    out_names = []
    out_avals = []
    zero_outs = []
    for alloc in nc.m.functions[0].allocations:
        if not isinstance(alloc, mybir.MemoryLocationSet):
            continue
        name = alloc.memorylocations[0].name
        if alloc.kind == "ExternalInput":
            if name != partition_name:
                in_names.append(name)
        elif alloc.kind == "ExternalOutput":
            out_names.append(name)
            shape = tuple(alloc.tensor_shape)
            dtype = mybir.dt.np(alloc.dtype)
            out_avals.append(jax.core.ShapedArray(shape, dtype))
            zero_outs.append(np.zeros(shape, dtype))
    n_params = len(in_names)
    n_outs = len(out_avals)
    all_names = in_names + out_names
    if partition_name is not None:
        all_names = all_names + [partition_name]
    donate = tuple(range(n_params, n_params + n_outs))

    def _body(*args):
        operands = list(args)
        if partition_name is not None:
            operands.append(bass2jax.partition_id_tensor())
        outs = bass2jax._bass_exec_p.bind(
            *operands,
            out_avals=tuple(out_avals),
            in_names=tuple(all_names),
            out_names=tuple(out_names),
            lowering_input_output_aliases=(),
            sim_require_finite=True,
            sim_require_nnan=True,
            nc=nc,
        )
        return tuple(outs)

    devices = jax.devices()[:N_CORES]
    mesh = Mesh(np.asarray(devices), ("core",))
    sharded = jax.jit(
        shard_map(
            _body,
            mesh=mesh,
            in_specs=(PartitionSpec("core"),) * (n_params + n_outs),
            out_specs=(PartitionSpec("core"),) * n_outs,
            check_rep=False,
        ),
        donate_argnums=donate,
        keep_unused=True,
    )

    def prep_args(in_maps):
        concat_in = [
            np.concatenate([np.asarray(in_maps[c][nm]) for c in range(N_CORES)], axis=0)
            for nm in in_names
        ]
        concat_zero = [np.concatenate([z] * N_CORES, axis=0) for z in zero_outs]
        return concat_in, concat_zero

    def run(in_maps):
        concat_in, concat_zero = prep_args(in_maps)
        out_arrs = sharded(*concat_in, *concat_zero)
        full = np.asarray(out_arrs[0])
        return np.split(full, N_CORES, axis=0)

    run.sharded = sharded
    run.prep_args = prep_args
    run.mesh = mesh
    run.nc = nc
    _RUNNER[t_steps] = run
    return run


def kernel(**inputs):
    inp = {k: np.asarray(v) for k, v in inputs.items()}
    for k in ("W_ih0", "W_hh0", "b_ih0", "b_hh0", "W_ih1", "W_hh1", "b_ih1",
              "b_hh1", "W_out", "b_out", "h0", "c0", "outputs"):
        assert k in inp, f"missing input {k}"

    shared = _prep_shared(inp)
    in_maps = []
    for c in range(N_CORES):
        m = dict(shared)
        m.update(_prep_core(inp, c))
        in_maps.append(m)

    run = _get_runner(_T)
    outs = run(in_maps)  # list of [BL, T_FULL] fp32 per core

    out_all = np.concatenate(outs[:ACTIVE], axis=0)  # [B, T_FULL]
    targets = np.asarray(inp["outputs"]).astype(np.float32)  # [B, T]
    d = out_all[:, :_T].astype(np.float64) - targets[:, :_T].astype(np.float64)
    loss = np.sum(np.mean(d * d, axis=0))
    return np.float32(loss)
